# revision 10
# baseline (speedup 1.0000x reference)
"""Distortion-loss (eff_distloss) Bass kernel for Trainium2, 8 NeuronCores.

Inputs (full): weights/distances/intervals, each [262144, 128] f32.
Output: scalar f32 loss.

Math: per ray (w, m, s in R^128):
  uni = sum_j s_j w_j^2
  bi  = sum_{j>k} w_j w_k (m_j - m_k) = wm^T (SL - SU) w,  wm = w*m,
        SL/SU strictly lower/upper triangular ones.
  loss = 0.01 * mean_rays(uni/3 + 2*bi)

Total bi over a batch of rays = <A^T, W^T WM>_F with A = SL - SU (constant)
and W^T WM a Gram matrix accumulated over rays; uni = sum diag(W^T SW),
sw = s*w. On the PE, each 128-ray block is ONE ldweights (stationary w) +
ONE 256-wide matmul streaming [wm ; sw] (a [P,2,N] strided rhs over the
slot's contiguous wm/sw halves) into a single [128, 256] PSUM accumulator
holding both Gram matrices. The 2x (bi) and 1/3 (uni) loss weights are
folded into the constant matrix, so the finale is one multiply+reduce
into a [128,1] column, a ones-column matmul collapsing it to a single
scalar, and a 4-byte store (a [128,x] store fans into 128 tiny
descriptors whose 16 per-engine completion incs straggle ~2us).

Sharding: pure data-parallel over the ray axis, B=262144 -> 32768 rays on
each of the 8 cores; the host sums the 8 scalars.

Engine split (raw bass, no Tile):
  gpsimd : w loads as SWDGE cast-DMAs (f32 HBM -> bf16 SBUF). Same HBM
           read bytes, but no DVE cast op -- DVE per-tile work drops from
           ~0.89x of the stream rate to ~0.71x, which is what lets the
           tail drain instead of bunching.
  sync   : m/s loads (HWDGE), the final 4-byte store.
  scalar : the constant-matrix load on the second HWDGE queue.
  vector : wm/sw products (bf16 w times f32 m/s -> bf16), the finale.
  tensor : Gram matmuls, the ones-column scalar collapse.
DMA completion uses one counting semaphore per (tensor, ring slot) with
full-transfer thresholds so the DVE starts wm the moment w+m land. The
last 4 tiles are 4 blocks each in dedicated (non-ring) buffers issued
with no ring guards, so the queue never starves at the tail and the
post-stream chain is a couple of small ops + finale."""

import numpy as np

import concourse.bass as bass
import concourse.mybir as mybir
from concourse.bass_utils import run_bass_kernel_spmd

B, N = 262144, 128
NCORES = 8
B_PER = B // NCORES  # 32768 rays per core
P = 128  # SBUF partitions = rays per matmul block
RMAX = 16  # rays per partition in a full tile
# 15 full ring tiles + dedicated tail tiles (progressively smaller)
SCHED = [16] * 15 + [4, 4, 4, 2, 2]
assert sum(SCHED) * P == B_PER
T = len(SCHED)
NRING = 15  # tiles that live in the ring
FREE = RMAX * N  # ring slot size (f32 elements per partition)
NB = 4  # ring depth
TBLK = sum(SCHED[NRING:])  # 16 tail blocks
TFREE = TBLK * N

F32 = mybir.dt.float32
BF16 = mybir.dt.bfloat16

LOSS_WEIGHT = 0.01

_cached = {}


def _build_nc() -> bass.Bass:
    nc = bass.Bass(trn_type="TRN2", monotonic_sem_count=0)

    w_h = nc.declare_dram_parameter("weights", [B_PER, N], F32, isOutput=False)
    m_h = nc.declare_dram_parameter("distances", [B_PER, N], F32, isOutput=False)
    s_h = nc.declare_dram_parameter("intervals", [B_PER, N], F32, isOutput=False)
    ai_h = nc.declare_dram_parameter("aimat", [P, 2 * N + 1], F32, isOutput=False)
    out_h = nc.declare_dram_parameter("partials", [1, 1], F32, isOutput=True)

    # per-tile DRAM views: tile i covers rays [off, off + P*R_i)
    offs = [0]
    for r in SCHED:
        offs.append(offs[-1] + P * r)

    def dram_view(h, i):
        r = SCHED[i]
        return h[offs[i] : offs[i + 1], :].rearrange("(p r) n -> p (r n)", p=P, r=r)

    # tail tile j starts at block toff[j] of the tail buffers
    toff = [0]
    for r in SCHED[NRING:]:
        toff.append(toff[-1] + r)

    # DVE inc ledger: wm+sw per tile -> 2 each, then finale reduce, then
    # the psum-scalar copy.
    def dve_after_tile(i):
        return 2 * (i + 1)

    DVE_RED = 2 * T + 1  # 39
    DVE_FINAL = DVE_RED + 1  # 40

    PE_ALL = T  # 19
    PE_SCALAR = T + 1  # 20

    import contextlib

    with contextlib.ExitStack() as ctx:
        ec = ctx.enter_context
        wb_sb = ec(nc.sbuf_tensor([P, NB * FREE], BF16))
        m_sb = ec(nc.sbuf_tensor([P, NB * FREE], F32))
        s_sb = ec(nc.sbuf_tensor([P, NB * FREE], F32))
        # slot k holds wm in [k*2F, k*2F+F) and sw in [k*2F+F, k*2F+2F),
        # both contiguous; the matmul rhs is a [P, 2, N] strided view
        ws_sb = ec(nc.sbuf_tensor([P, NB * 2 * FREE], BF16))
        # dedicated tail buffers (no ring reuse, no guards)
        wb_tl = ec(nc.sbuf_tensor([P, TFREE], BF16))
        m_tl = ec(nc.sbuf_tensor([P, TFREE], F32))
        s_tl = ec(nc.sbuf_tensor([P, TFREE], F32))
        ws_tl = ec(nc.sbuf_tensor([P, 2 * TFREE], BF16))
        ai_sb = ec(nc.sbuf_tensor([P, 2 * N + 1], F32))
        acc_sb = ec(nc.sbuf_tensor([P, 1], F32))
        outs_sb = ec(nc.sbuf_tensor([1, 1], F32))
        tr_sb = ec(nc.sbuf_tensor([P, 2 * N], F32))
        g12_ps = ec(nc.psum_tensor([P, 2 * N], F32))  # [W^T WM | W^T SW]
        sc_ps = ec(nc.psum_tensor([1, 1], F32))
        w_sl = [ec(nc.semaphore(f"dma_w{i}")) for i in range(NB)]
        m_sl = [ec(nc.semaphore(f"dma_m{i}")) for i in range(NB)]
        s_sl = [ec(nc.semaphore(f"dma_s{i}")) for i in range(NB)]
        NT = T - NRING
        w_tsem = [ec(nc.semaphore(f"dma_wt{j}")) for j in range(NT)]
        m_tsem = [ec(nc.semaphore(f"dma_mt{j}")) for j in range(NT)]
        s_tsem = [ec(nc.semaphore(f"dma_st{j}")) for j in range(NT)]
        ai_sem = ec(nc.semaphore("dma_ai"))
        dve_sem = ec(nc.semaphore("dve_sem"))
        pe_sem = ec(nc.semaphore("pe_sem"))
        block = ec(nc.Block(no_gpsimd_drain=True))

        def rnd(i):
            # full-transfer threshold for ring tile i on its slot sem
            return 16 * (i // NB + 1)

        def sl(i):
            base = (i % NB) * FREE
            return slice(base, base + SCHED[i] * N)

        def tsl(i):
            j = i - NRING
            return slice(toff[j] * N, toff[j + 1] * N)

        def wm_dst(i):
            if i < NRING:
                base = (i % NB) * 2 * FREE
                return ws_sb[:, base : base + SCHED[i] * N]
            j = i - NRING
            return ws_tl[:, toff[j] * N : toff[j + 1] * N]

        def sw_dst(i):
            if i < NRING:
                base = (i % NB) * 2 * FREE + FREE
                return ws_sb[:, base : base + SCHED[i] * N]
            j = i - NRING
            return ws_tl[:, TFREE + toff[j] * N : TFREE + toff[j + 1] * N]

        def rhs_blk(i, r):
            # [P, 2, N] strided view: (wm_r ; sw_r) of block r
            if i < NRING:
                base2 = (i % NB) * 2 * FREE
                v = ws_sb[:, base2 : base2 + 2 * FREE].rearrange(
                    "p (two f) -> p two f", two=2
                )
            else:
                v = ws_tl[:].rearrange("p (two f) -> p two f", two=2)
                r = toff[i - NRING] + r
            return v[:, :, r * N : (r + 1) * N]

        def lhs_blk(i, r):
            if i < NRING:
                base = (i % NB) * FREE
                return wb_sb[:, base + r * N : base + (r + 1) * N]
            r = toff[i - NRING] + r
            return wb_tl[:, r * N : (r + 1) * N]

        @block.scalar
        def _(act: bass.BassEngine):
            # constants ride the second HWDGE queue: off the sync queue's
            # FIFO, lands at stream start
            act.dma_start(out=ai_sb[:], in_=ai_h[:, :]).then_inc(ai_sem, 16)

        @block.gpsimd
        def _(g: bass.BassEngine):
            # w rides the SWDGE queue as cast-DMAs (f32 -> bf16): same HBM
            # reads, no DVE cast op, and w needs only half the SBUF
            for i in range(NRING):
                if i >= NB:
                    # wb ring slot (i-NB) fully consumed by the PE
                    g.wait_ge(pe_sem, i - NB + 1)
                g.dma_start(out=wb_sb[:, sl(i)], in_=dram_view(w_h, i)).then_inc(
                    w_sl[i % NB], 16
                )
            for i in range(NRING, T):
                g.dma_start(out=wb_tl[:, tsl(i)], in_=dram_view(w_h, i)).then_inc(
                    w_tsem[i - NRING], 16
                )

        @block.sync
        def _(sync: bass.BassEngine):
            for i in range(NRING):
                k = i % NB
                if i >= NB:
                    # m/s ring slot (i-NB) fully consumed by DVE
                    sync.wait_ge(dve_sem, dve_after_tile(i - NB))
                sync.dma_start(out=m_sb[:, sl(i)], in_=dram_view(m_h, i)).then_inc(
                    m_sl[k], 16
                )
                sync.dma_start(out=s_sb[:, sl(i)], in_=dram_view(s_h, i)).then_inc(
                    s_sl[k], 16
                )
            for i in range(NRING, T):
                sync.dma_start(out=m_tl[:, tsl(i)], in_=dram_view(m_h, i)).then_inc(
                    m_tsem[i - NRING], 16
                )
                sync.dma_start(out=s_tl[:, tsl(i)], in_=dram_view(s_h, i)).then_inc(
                    s_tsem[i - NRING], 16
                )
            sync.wait_ge(dve_sem, DVE_FINAL)
            sync.dma_start(out=out_h[:, :], in_=outs_sb[:]).then_inc(pe_sem, 16)
            # the out-DMA must fully land before the NEFF ends: an in-flight
            # DMA across the NEFF boundary corrupts runtime state.
            sync.wait_ge(pe_sem, PE_SCALAR + 16)

        @block.vector
        def _(vector: bass.BassEngine):
            for i in range(NRING):
                k = i % NB
                if i >= NB:
                    # bf16 ws ring slot (i-NB) fully consumed by PE
                    vector.wait_ge(pe_sem, i - NB + 1)
                vector.wait_ge(w_sl[k], rnd(i))
                vector.wait_ge(m_sl[k], rnd(i))
                vector.tensor_mul(wm_dst(i), wb_sb[:, sl(i)], m_sb[:, sl(i)]).then_inc(
                    dve_sem, 1
                )
                vector.wait_ge(s_sl[k], rnd(i))
                vector.tensor_mul(sw_dst(i), s_sb[:, sl(i)], wb_sb[:, sl(i)]).then_inc(
                    dve_sem, 1
                )
            for i in range(NRING, T):
                j = i - NRING
                # one sem per (tensor, tail tile): a threshold can never be
                # crossed by a later transfer's per-lane incs
                vector.wait_ge(w_tsem[j], 16)
                vector.wait_ge(m_tsem[j], 16)
                vector.tensor_mul(wm_dst(i), wb_tl[:, tsl(i)], m_tl[:, tsl(i)]).then_inc(
                    dve_sem, 1
                )
                vector.wait_ge(s_tsem[j], 16)
                vector.tensor_mul(sw_dst(i), s_tl[:, tsl(i)], wb_tl[:, tsl(i)]).then_inc(
                    dve_sem, 1
                )
            # finale: (G * [2A | I/3]) multiply-reduce -> [128,1]
            # (tensor_tensor_reduce would fuse these but fails codegen)
            vector.wait_ge(pe_sem, PE_ALL)
            vector.wait_ge(ai_sem, 16)
            vector.tensor_mul(tr_sb[:], g12_ps[:], ai_sb[:, 0 : 2 * N])
            vector.tensor_reduce(
                acc_sb[:],
                tr_sb[:],
                axis=mybir.AxisListType.X,
                op=mybir.AluOpType.add,
            ).then_inc(dve_sem, 1)
            # collapse to one scalar via the PE, then stage it for the DMA
            vector.wait_ge(pe_sem, PE_SCALAR)
            vector.tensor_copy(out=outs_sb[:], in_=sc_ps[:]).then_inc(dve_sem, 1)

        @block.tensor
        def _(tensor: bass.BassEngine):
            for i in range(T):
                # one matmul per ray block; the tile's wm+sw must be done
                tensor.wait_ge(dve_sem, 2 * i + 2)
                last_mm = None
                for r in range(SCHED[i]):
                    last_mm = nc.tensor.matmul(
                        out=g12_ps[:],
                        lhsT=lhs_blk(i, r),
                        rhs=rhs_blk(i, r),
                        start=(i == 0 and r == 0),
                        stop=(i == T - 1 and r == SCHED[i] - 1),
                    )
                last_mm.then_inc(pe_sem, 1)
            # ones-weighted column sum: [1,1] scalar in PSUM
            tensor.wait_ge(dve_sem, DVE_RED)
            nc.tensor.matmul(
                out=sc_ps[:],
                lhsT=acc_sb[:],
                rhs=ai_sb[:, 2 * N : 2 * N + 1],
                start=True,
                stop=True,
            ).then_inc(pe_sem, 1)

    return nc


def _aimat() -> np.ndarray:
    # transpose of (SL - SU): the kernel accumulates W^T WM = G1^T, and
    # <A, G1> = <A^T, G1^T>. The 2x (bi) and 1/3 (uni) loss weights are
    # folded in; the trailing column of ones drives the scalar-collapse
    # matmul.
    a = np.triu(np.ones((N, N), np.float32), 1) - np.tril(
        np.ones((N, N), np.float32), -1
    )
    return np.ascontiguousarray(
        np.concatenate(
            [
                2.0 * a,
                np.eye(N, dtype=np.float32) / 3.0,
                np.ones((N, 1), dtype=np.float32),
            ],
            axis=1,
        )
    )


def kernel(weights: np.ndarray, distances: np.ndarray, intervals: np.ndarray):
    if "nc" not in _cached:
        _cached["nc"] = _build_nc()
    nc = _cached["nc"]

    w8 = np.ascontiguousarray(weights, np.float32).reshape(NCORES, B_PER, N)
    m8 = np.ascontiguousarray(distances, np.float32).reshape(NCORES, B_PER, N)
    s8 = np.ascontiguousarray(intervals, np.float32).reshape(NCORES, B_PER, N)
    ai = _aimat()

    in_maps = [
        {
            "weights": w8[i],
            "distances": m8[i],
            "intervals": s8[i],
            "aimat": ai,
        }
        for i in range(NCORES)
    ]
    res = run_bass_kernel_spmd(nc, in_maps, list(range(NCORES))).results

    total = 0.0
    for i in range(NCORES):
        total += float(res[i]["partials"].astype(np.float64)[0, 0])

    loss = LOSS_WEIGHT * total / B
    return np.asarray(loss, dtype=np.float32)
